# revision 21
# baseline (speedup 1.0000x reference)
"""DetectionBEVLoss Trainium2 kernel: 8-core data-parallel (1 batch/core).

Per core 65536 elements as [128 partitions, 512 free], full-width ops.
Rotated IoU via branch-free Liang-Barsky edge clipping with closed-form
edge directions (edge dirs of a rotated rect are +-2*{cos,sin}(dyaw)*halfdim,
so the clip reciprocals come straight from the trig products - no corner
differencing) and a closed-form A-side cross-product sum:
  sum_e CR_e*S_e = 2*lhp*dcy2*(S0-S2) + 2*whp*dcx2*(S1-S3) - 2*lhp*whp*sum(S_A)
Engines: DVE does the TT-heavy geometry; ACT (single table set:
natural_log_exp_and_others = abs/relu/square/exp/ln/copy) takes the
activations; gpsimd is kept off the critical path. All 9 loss partial sums
are fused multiply+reduce (tensor_tensor_reduce) into one fp32 accumulator,
cross-partition reduced by one TensorE matmul.
Input DMA is staged in 3 pieces (geometry slots first) so compute starts
~4us in instead of waiting for the full 4MiB.
"""
import numpy as np

import concourse.bacc as bacc
import concourse.bass as bass
import concourse.mybir as mybir
import concourse.tile as tile
from concourse.bass_utils import run_bass_kernel_spmd

F16 = mybir.dt.float16
F32 = mybir.dt.float32
OP = mybir.AluOpType
AF = mybir.ActivationFunctionType

P = 128
FW = 512
EPS = 1e-7

# IN1 slots: 0 yawp, 1 yawt, 2 wp, 3 lp, 4 wt, 5 lt, 6 xp, 7 yp, 8 xt, 9 yt
# IN2 slots: 0 zp, 1 zt, 2 hp, 3 ht, 4 vxp, 5 vxt, 6 vyp, 7 vyt,
#            8 ioup, 9 iout, 10 clst, 11 w
# IN3 slots: cls_pred c0..c9


def _ap(t, s0, slot_dims, col0=0, ncol=FW, colstep=1):
    """Manual AP into tile t ([128, S, W]): base slot s0, then
    (slot_step, count) dims, innermost column dim."""
    ss = t.ap[-2][0]
    ap = [list(t.ap[0])] + [[s * ss, c] for s, c in slot_dims] + [[colstep, ncol]]
    return bass.AP(tensor=t.tensor, offset=t.offset + s0 * ss + col0, ap=ap)


DBG_SLOTS = 64


def build_bass(dbg=False):
    nc = bacc.Bacc("TRN2", target_bir_lowering=False, debug=False)
    in1a = nc.declare_dram_parameter("in1a", [P, 2, FW], F16, isOutput=False)
    in1b = nc.declare_dram_parameter("in1b", [P, 8, FW], F16, isOutput=False)
    in2 = nc.declare_dram_parameter("in2", [P, 12, FW], F16, isOutput=False)
    in3 = nc.declare_dram_parameter("in3", [P, 10, FW], F16, isOutput=False)
    outp = nc.declare_dram_parameter("out", [1, 16], F32, isOutput=True)
    dbg_slots = {}
    if dbg:
        dbgp = nc.declare_dram_parameter("dbg", [P, DBG_SLOTS, FW], F16,
                                         isOutput=True)
        dbg_next = [0]

        def tap(name, t, k):
            s = dbg_next[0]
            assert s + k <= DBG_SLOTS
            nc.sync.dma_start(out=dbgp[:, s:s + k, :], in_=t)
            dbg_slots[name] = (s, k)
            dbg_next[0] += k
    else:
        def tap(name, t, k):
            pass

    with tile.TileContext(nc) as tc:
        with (
            tc.tile_pool(name="main", bufs=1) as pool,
            tc.tile_pool(name="small", bufs=1) as spool,
            tc.tile_pool(name="ps", bufs=1, space="PSUM") as ppool,
        ):
            IN1A = pool.tile([P, 2, FW], F16)
            IN1B = pool.tile([P, 8, FW], F16)
            IN2 = pool.tile([P, 12, FW], F16)
            IN3 = pool.tile([P, 10, FW], F16)
            nc.sync.dma_start(out=IN1A, in_=in1a[:, :, :])
            nc.sync.dma_start(out=IN1B, in_=in1b[:, :, :])
            nc.sync.dma_start(out=IN2, in_=in2[:, :, :])
            nc.sync.dma_start(out=IN3, in_=in3[:, :, :])

            ones = spool.tile([P, 1], F32)
            nc.vector.memset(ones, 1.0)
            ACC = spool.tile([P, 16], F32)
            nc.vector.memset(ACC, 0.0)
            JUNK = pool.tile([P, FW], F16, tag="JUNK")

            def acc_sum(in0, in1_, col, scale=1.0, out=None):
                # fused (in0*scale)*in1 with free-dim sum into ACC[:, col]
                nc.vector.scalar_tensor_tensor(
                    out=out if out is not None else JUNK,
                    in0=in0, scalar=scale, in1=in1_,
                    op0=OP.mult, op1=OP.mult,
                    accum_out=ACC[:, col:col + 1])

            # ================= trig (needs IN1 only) =================

            PIB = spool.tile([P, 1], F32)
            nc.vector.memset(PIB, 1.5707963267948966)
            TR = pool.tile([P, 4, FW], F16)       # [cp, sp, ct, st]
            nc.scalar.activation(_ap(TR, 1, [(2, 2)]), IN1A, AF.Sin)
            nc.scalar.activation(_ap(TR, 0, [(2, 2)]), IN1A, AF.Sin,
                                 bias=PIB[:, 0:1])
            tap("TR", TR, 4)
            ACS = pool.tile([P, 4, FW], F16)      # |cp| |sp| |ct| |st|
            nc.scalar.activation(ACS, TR, AF.Abs)

            # TP=[cp*ct, sp*st], TQ=[sp*ct, cp*st]
            TP = pool.tile([P, 2, FW], F16, tag="T2a")
            TQ = pool.tile([P, 2, FW], F16, tag="T2b")
            nc.vector.tensor_tensor(out=TP, in0=TR[:, 0:2, :], in1=TR[:, 2:4, :],
                                    op=OP.mult)
            nc.vector.tensor_tensor(out=TQ, in0=_ap(TR, 1, [(-1, 2)]),
                                    in1=TR[:, 2:4, :], op=OP.mult)
            # CS8 = [-cd,-sd,-sd,+cd,+cd,-sd,-sd,-cd]; cd=TP0+TP1, sd=TQ0-TQ1
            CS8 = pool.tile([P, 8, FW], F16, tag="S8a")
            nc.vector.tensor_tensor(out=_ap(CS8, 3, [(1, 2)]),
                                    in0=_ap(TP, 0, [(0, 2)]),
                                    in1=_ap(TP, 1, [(0, 2)]), op=OP.add)
            nc.vector.scalar_tensor_tensor(out=_ap(CS8, 0, [(7, 2)]),
                                           in0=_ap(TP, 0, [(0, 2)]), scalar=-1.0,
                                           in1=_ap(TP, 1, [(0, 2)]),
                                           op0=OP.mult, op1=OP.subtract)
            nc.vector.tensor_tensor(out=_ap(CS8, 1, [(4, 2), (1, 2)]),
                                    in0=_ap(TQ, 1, [(0, 2), (0, 2)]),
                                    in1=_ap(TQ, 0, [(0, 2), (0, 2)]),
                                    op=OP.subtract)

            HV = pool.tile([P, 4, FW], F16)       # [lht, wht, lhp, whp]
            nc.vector.tensor_scalar(out=HV, in0=_ap(IN1B, 3, [(-1, 4)]),
                                    scalar1=0.5, scalar2=None, op0=OP.mult)
            DXY = pool.tile([P, 2, FW], F16)      # [dx, dy]
            nc.vector.tensor_tensor(out=DXY, in0=IN1B[:, 4:6, :],
                                    in1=IN1B[:, 6:8, :], op=OP.subtract)
            tap("HV", HV, 4)
            tap("DXY", DXY, 2)
            # DC = [dcx, dcy, dcx2, dcy2]
            PT = pool.tile([P, 4, FW], F16, tag="S4a")
            QT = pool.tile([P, 4, FW], F16, tag="S4b")
            nc.vector.tensor_tensor(out=PT, in0=_ap(DXY, 0, [(0, 2), (1, 2)]),
                                    in1=_ap(TR, 2, [(-2, 2), (0, 2)]), op=OP.mult)
            nc.vector.tensor_tensor(out=QT, in0=_ap(DXY, 0, [(0, 2), (1, 2)]),
                                    in1=_ap(TR, 3, [(-2, 2), (0, 2)]), op=OP.mult)
            DC = pool.tile([P, 4, FW], F16)
            nc.vector.tensor_tensor(out=_ap(DC, 0, [(2, 2)]),
                                    in0=_ap(PT, 0, [(2, 2)]),
                                    in1=_ap(QT, 1, [(2, 2)]), op=OP.add)
            nc.vector.tensor_tensor(out=_ap(DC, 1, [(2, 2)]),
                                    in0=_ap(PT, 1, [(2, 2)]),
                                    in1=_ap(QT, 0, [(2, 2)]), op=OP.subtract)

            tap("DC", DC, 4)
            # UVXD = CS8 * [lhp,whp,lhp,whp,lht,wht,lht,wht]  (= D/2)
            UVXD = pool.tile([P, 8, FW], F16, tag="S8b")
            nc.vector.tensor_tensor(out=UVXD, in0=CS8,
                                    in1=_ap(HV, 2, [(-2, 2), (0, 2), (1, 2)]),
                                    op=OP.mult)

            tap("UVXD", UVXD, 8)
            # SC8 = [sA, sC, sB, sD, pA, pB, pC, pD]
            SC8 = pool.tile([P, 8, FW], F16, tag="S8c")
            nc.vector.scalar_tensor_tensor(out=_ap(SC8, 0, [(1, 2)]),
                                           in0=_ap(UVXD, 0, [(2, 2)]), scalar=-1.0,
                                           in1=_ap(UVXD, 1, [(2, 2)]),
                                           op0=OP.mult, op1=OP.subtract)
            nc.vector.tensor_tensor(out=_ap(SC8, 2, [(1, 2)]),
                                    in0=_ap(UVXD, 1, [(2, 2)]),
                                    in1=_ap(UVXD, 0, [(2, 2)]), op=OP.subtract)
            nc.vector.tensor_tensor(out=SC8[:, 4, :], in0=UVXD[:, 4, :],
                                    in1=UVXD[:, 5, :], op=OP.add)
            nc.vector.tensor_tensor(out=SC8[:, 5, :], in0=UVXD[:, 4, :],
                                    in1=UVXD[:, 5, :], op=OP.subtract)
            nc.vector.scalar_tensor_tensor(out=SC8[:, 6, :], in0=UVXD[:, 6, :],
                                           scalar=-1.0, in1=UVXD[:, 7, :],
                                           op0=OP.mult, op1=OP.subtract)
            nc.vector.tensor_tensor(out=SC8[:, 7, :], in0=UVXD[:, 7, :],
                                    in1=UVXD[:, 6, :], op=OP.subtract)

            tap("SC8", SC8, 8)
            # corners: CRN = [AX0..3, AY0..3, BX0..3, BY0..3]
            CRN = pool.tile([P, 16, FW], F16, tag="B16a")

            def corner2(dst0, dstep, dcslot, scslot, scstep, op):
                nc.vector.tensor_tensor(
                    out=_ap(CRN, dst0, [(dstep, 2)]),
                    in0=_ap(DC, dcslot, [(0, 2)]),
                    in1=_ap(SC8, scslot, [(scstep, 2)]), op=op)

            corner2(0, 3, 0, 0, 2, OP.add)        # AX0=dcx+sA, AX3=dcx+sB
            corner2(1, 1, 0, 2, -2, OP.subtract)  # AX1=dcx-sB, AX2=dcx-sA
            corner2(4, 3, 1, 1, 2, OP.add)        # AY0=dcy+sC, AY3=dcy+sD
            corner2(5, 1, 1, 3, -2, OP.subtract)  # AY1=dcy-sD, AY2=dcy-sC
            corner2(8, 3, 2, 4, 1, OP.subtract)   # BX0=dcx2-pA, BX3=dcx2-pB
            corner2(9, 1, 2, 5, -1, OP.add)       # BX1=dcx2+pB, BX2=dcx2+pA
            corner2(12, 3, 3, 6, 1, OP.add)       # BY0=dcy2+pC, BY3=dcy2+pD
            corner2(13, 1, 3, 7, -1, OP.subtract) # BY1=dcy2-pD, BY2=dcy2-pC

            # focal front-end early: ACT exp + Pool trees run under geometry
            clsf = IN2[:, 10, :]
            ET = pool.tile([P, 10, FW], F16, tag="S10a")
            nc.scalar.activation(ET, IN3, AF.Exp)
            S5 = pool.tile([P, 5, FW], F16, tag="S5a")
            nc.gpsimd.tensor_tensor(out=S5, in0=ET[:, 0:5, :], in1=ET[:, 5:10, :],
                                    op=OP.add)
            # ============ clip: reciprocals from UVXD ============
            UVX32 = pool.tile([P, 8, FW], F32, tag="F32a")
            nc.scalar.activation(UVX32, UVXD, AF.Copy, bias=1e-12)
            REC32 = UVX32
            nc.vector.reciprocal_approx_fast(
                out=REC32.rearrange("p a b -> p (a b)"),
                in_=UVX32.rearrange("p a b -> p (a b)"))
            # r = REC/2 clamped to +-8000
            nc.vector.tensor_scalar(out=REC32, in0=REC32, scalar1=0.5,
                                    scalar2=-8000.0, op0=OP.mult, op1=OP.max)
            RD8 = pool.tile([P, 8, FW], F16, tag="S8b")
            nc.vector.tensor_scalar(out=RD8, in0=REC32, scalar1=8000.0,
                                    scalar2=None, op0=OP.min)
            tap("RD8", RD8, 8)
            RA = pool.tile([P, 8, FW], F16, tag="S8a")
            nc.scalar.activation(RA, RD8, AF.Abs)
            RL = pool.tile([P, 8, FW], F16, tag="S8c")
            nc.vector.tensor_tensor(out=RL, in0=RA,
                                    in1=_ap(HV, 0, [(1, 4), (0, 2)]), op=OP.mult)

            tap("RL", RL, 8)
            rep16 = [(2, 4), (0, 2), (1, 2)]
            P16 = pool.tile([P, 16, FW], F16, tag="B16b")
            nc.vector.tensor_tensor(out=P16, in0=CRN,
                                    in1=_ap(RD8, 0, rep16), op=OP.mult)
            OPA = pool.tile([P, 16, FW], F16, tag="B16a")   # reuse CRN buffer
            nc.vector.tensor_tensor(out=OPA, in0=_ap(RL, 0, rep16), in1=P16,
                                    op=OP.subtract)
            OPB = P16   # in-place: OPB = RL16rep + P16 overwrites P16
            nc.vector.tensor_tensor(out=OPB, in0=_ap(RL, 0, rep16), in1=P16,
                                    op=OP.add)

            # T1m/T0m: min over the two axes; edges {0,1} vs {2,3} swap A/B roles
            T1m = pool.tile([P, 8, FW], F16, tag="S8a")
            T0m = pool.tile([P, 8, FW], F16, tag="S8b")
            nc.vector.tensor_tensor(out=_ap(T1m, 0, [(4, 2), (1, 2)]),
                                    in0=_ap(OPA, 0, [(8, 2), (1, 2)]),
                                    in1=_ap(OPA, 4, [(8, 2), (1, 2)]), op=OP.min)
            nc.vector.tensor_tensor(out=_ap(T1m, 2, [(4, 2), (1, 2)]),
                                    in0=_ap(OPB, 2, [(8, 2), (1, 2)]),
                                    in1=_ap(OPB, 6, [(8, 2), (1, 2)]), op=OP.min)
            nc.vector.tensor_tensor(out=_ap(T0m, 0, [(4, 2), (1, 2)]),
                                    in0=_ap(OPB, 0, [(8, 2), (1, 2)]),
                                    in1=_ap(OPB, 4, [(8, 2), (1, 2)]), op=OP.min)
            nc.vector.tensor_tensor(out=_ap(T0m, 2, [(4, 2), (1, 2)]),
                                    in0=_ap(OPA, 2, [(8, 2), (1, 2)]),
                                    in1=_ap(OPA, 6, [(8, 2), (1, 2)]), op=OP.min)
            # SEG = relu(min(T1,1) - relu(-T0m))
            nc.scalar.activation(T0m, T0m, AF.Relu, scale=-1.0)
            nc.vector.tensor_scalar(out=T1m, in0=T1m, scalar1=1.0,
                                    scalar2=None, op0=OP.min)
            SEG = pool.tile([P, 8, FW], F16, tag="S8c")
            nc.vector.tensor_tensor(out=SEG, in0=T1m, in1=T0m, op=OP.subtract)
            nc.scalar.activation(SEG, SEG, AF.Relu)

            tap("SEG", SEG, 8)
            # ============ intersection (closed-form cross sum) ============
            SD1 = pool.tile([P, 2, FW], F16, tag="T2a")   # [S0-S2, S1-S3]
            nc.vector.tensor_tensor(out=SD1, in0=_ap(SEG, 0, [(1, 2)]),
                                    in1=_ap(SEG, 2, [(1, 2)]), op=OP.subtract)
            SALL = pool.tile([P, 4, FW], F16, tag="S4a")  # [S0+S2,S1+S3,S4+S6,S5+S7]
            nc.vector.tensor_tensor(out=SALL, in0=_ap(SEG, 0, [(4, 2), (1, 2)]),
                                    in1=_ap(SEG, 2, [(4, 2), (1, 2)]), op=OP.add)
            SS2 = pool.tile([P, 2, FW], F16, tag="T2b")   # [sumS_A, sumS_B]
            nc.vector.tensor_tensor(out=SS2, in0=_ap(SALL, 0, [(2, 2)]),
                                    in1=_ap(SALL, 1, [(2, 2)]), op=OP.add)
            Pm = pool.tile([P, 2, FW], F16, tag="T2c")    # [dcy2*SD0, dcx2*SD1]
            nc.vector.tensor_tensor(out=Pm, in0=SD1,
                                    in1=_ap(DC, 3, [(-1, 2)]), op=OP.mult)
            nc.vector.tensor_tensor(out=Pm, in0=Pm,
                                    in1=_ap(HV, 2, [(1, 2)]), op=OP.mult)
            AREA2 = pool.tile([P, 2, FW], F16, tag="A2")  # [lhp*whp, lht*wht]
            nc.vector.tensor_tensor(out=AREA2, in0=_ap(HV, 2, [(-2, 2)]),
                                    in1=_ap(HV, 3, [(-2, 2)]), op=OP.mult)
            MM2 = pool.tile([P, 2, FW], F16, tag="T2d")
            nc.vector.tensor_tensor(out=MM2, in0=AREA2, in1=SS2, op=OP.mult)
            nc.vector.tensor_tensor(out=Pm, in0=Pm, in1=MM2, op=OP.subtract)
            HACA = pool.tile([P, FW], F16, tag="K1")
            nc.vector.tensor_tensor(out=HACA, in0=Pm[:, 0, :], in1=Pm[:, 1, :],
                                    op=OP.add)
            INTER = pool.tile([P, FW], F16, tag="K2")
            nc.scalar.activation(INTER, HACA, AF.Abs)

            tap("INTER", INTER, 1)
            U1 = pool.tile([P, FW], F16, tag="K3")
            nc.vector.tensor_tensor(out=U1, in0=AREA2[:, 0, :],
                                    in1=AREA2[:, 1, :], op=OP.add)
            UNION = pool.tile([P, FW], F16, tag="K4")
            nc.vector.scalar_tensor_tensor(out=UNION, in0=U1, scalar=4.0,
                                           in1=INTER, op0=OP.mult, op1=OP.subtract)
            ING = pool.tile([P, FW], F16, tag="K5")
            nc.vector.scalar_tensor_tensor(out=ING, in0=UNION, scalar=EPS,
                                           in1=INTER, op0=OP.is_gt, op1=OP.mult)
            UC = UNION
            nc.vector.tensor_scalar(out=UC, in0=UNION, scalar1=EPS,
                                    scalar2=None, op0=OP.max)

            # ============ enclosing box + center dist ============
            PA_ = pool.tile([P, 4, FW], F16, tag="S4a")
            PB_ = pool.tile([P, 4, FW], F16, tag="S4b")
            nc.vector.tensor_tensor(out=PA_, in0=_ap(HV, 2, [(-2, 2), (1, 2)]),
                                    in1=ACS, op=OP.mult)
            nc.vector.tensor_tensor(out=PB_, in0=_ap(HV, 2, [(-2, 2), (1, 2)]),
                                    in1=_ap(ACS, 1, [(2, 2), (-1, 2)]), op=OP.mult)
            E2 = pool.tile([P, 4, FW], F16, tag="S4c")    # [exP, exT, eyP, eyT]
            nc.vector.tensor_tensor(out=_ap(E2, 0, [(1, 2)]),
                                    in0=_ap(PA_, 0, [(2, 2)]),
                                    in1=_ap(PA_, 1, [(2, 2)]), op=OP.add)
            nc.vector.tensor_tensor(out=_ap(E2, 2, [(1, 2)]),
                                    in0=_ap(PB_, 0, [(2, 2)]),
                                    in1=_ap(PB_, 1, [(2, 2)]), op=OP.add)
            # Earr = [exP, eyP, exT, eyT]; CEN = [xp, yp, xt, yt]
            XE = pool.tile([P, 4, FW], F16, tag="S4d")
            XD = pool.tile([P, 4, FW], F16, tag="S4e")
            # Earr: slots (0,2,1,3) of E2 => [exP, eyP, exT, eyT]
            Earr = _ap(E2, 0, [(1, 2), (2, 2)])
            nc.vector.tensor_tensor(out=XE, in0=IN1B[:, 4:8, :], in1=Earr,
                                    op=OP.add)
            nc.vector.tensor_tensor(out=XD, in0=IN1B[:, 4:8, :], in1=Earr,
                                    op=OP.subtract)
            HX = pool.tile([P, 2, FW], F16, tag="T2a")
            LX = pool.tile([P, 2, FW], F16, tag="T2b")
            nc.vector.tensor_tensor(out=HX, in0=_ap(XE, 0, [(1, 2)]),
                                    in1=_ap(XE, 2, [(1, 2)]), op=OP.max)
            nc.vector.tensor_tensor(out=LX, in0=_ap(XD, 0, [(1, 2)]),
                                    in1=_ap(XD, 2, [(1, 2)]), op=OP.min)
            W2 = pool.tile([P, 2, FW], F16, tag="T2c")
            nc.vector.tensor_tensor(out=W2, in0=HX, in1=LX, op=OP.subtract)
            SQ2 = pool.tile([P, 2, FW], F16, tag="T2d")
            nc.scalar.activation(SQ2, W2, AF.Square)
            C2 = pool.tile([P, FW], F16, tag="K8")
            nc.vector.tensor_tensor(out=C2, in0=SQ2[:, 0, :], in1=SQ2[:, 1, :],
                                    op=OP.add)
            nc.vector.tensor_scalar(out=C2, in0=C2, scalar1=EPS,
                                    scalar2=None, op0=OP.max)
            D2P = pool.tile([P, 2, FW], F16, tag="T2e")
            nc.scalar.activation(D2P, DXY, AF.Square)
            D2 = pool.tile([P, FW], F16, tag="K9")
            nc.vector.tensor_tensor(out=D2, in0=D2P[:, 0, :], in1=D2P[:, 1, :],
                                    op=OP.add)

            # DL = (d2*UC - ING*C2) / (C2*UC); one reciprocal
            CM = pool.tile([P, FW], F32, tag="KF1")
            nc.vector.tensor_tensor(out=CM, in0=C2, in1=UC, op=OP.mult)
            RECM = pool.tile([P, FW], F32, tag="KF2")
            nc.vector.reciprocal_approx_fast(out=RECM, in_=CM)
            N1 = D2
            nc.vector.tensor_tensor(out=N1, in0=D2, in1=UC, op=OP.mult)
            N2 = pool.tile([P, FW], F16, tag="K11")
            nc.vector.tensor_tensor(out=N2, in0=ING, in1=C2, op=OP.mult)
            nc.vector.tensor_tensor(out=N1, in0=N1, in1=N2, op=OP.subtract)
            DL = N1
            nc.vector.tensor_tensor(out=DL, in0=N1, in1=RECM, op=OP.mult)
            wm = IN2[:, 11, :]
            acc_sum(DL, wm, 2)

            tap("C2", C2, 1)
            tap("D2", D2, 1)
            tap("DL", DL, 1)
            # ============ smooth L1 (z, h, vx, vy) ============
            DD = pool.tile([P, 4, FW], F16, tag="S4a")
            nc.vector.tensor_tensor(out=DD, in0=_ap(IN2, 0, [(2, 4)]),
                                    in1=_ap(IN2, 1, [(2, 4)]), op=OP.subtract)
            AD = pool.tile([P, 4, FW], F16, tag="S4b")
            nc.scalar.activation(AD, DD, AF.Abs)
            RM = pool.tile([P, 4, FW], F16, tag="S4c")
            nc.scalar.activation(RM, AD, AF.Relu, scale=-1.0, bias=1.0)
            R2h = pool.tile([P, 4, FW], F16, tag="S4d")
            nc.scalar.activation(R2h, RM, AF.Square, scale=0.7071067811865476)
            SL = pool.tile([P, 4, FW], F16, tag="S4e")
            nc.vector.tensor_tensor(out=SL, in0=AD, in1=R2h, op=OP.add)
            for k in range(4):
                acc_sum(SL[:, k, :], wm, 3 + k)

            # ============ BCE on iou head ============
            iop = IN2[:, 8, :]
            BA = pool.tile([P, FW], F16, tag="K13")
            nc.scalar.activation(BA, iop, AF.Abs)
            nc.scalar.activation(BA, BA, AF.Exp, scale=-1.0)
            nc.scalar.activation(BA, BA, AF.Ln, bias=1.0)
            BR = pool.tile([P, FW], F16, tag="K14")
            nc.scalar.activation(BR, iop, AF.Relu)
            BXY = pool.tile([P, FW], F16, tag="K15")
            nc.vector.tensor_tensor(out=BXY, in0=iop, in1=IN2[:, 9, :],
                                    op=OP.mult)
            nc.vector.tensor_tensor(out=BR, in0=BR, in1=BXY, op=OP.subtract)
            nc.vector.tensor_tensor(out=BR, in0=BR, in1=BA, op=OP.add)
            acc_sum(BR, wm, 7)

            # ============ focal (tail) ============
            EQ10 = pool.tile([P, 10, FW], F16, tag="S10b")
            for c in range(10):
                nc.vector.tensor_scalar(out=EQ10[:, c, :], in0=clsf,
                                        scalar1=float(c), scalar2=None,
                                        op0=OP.is_equal)
            MT = pool.tile([P, 10, FW], F16, tag="S10a")   # reuse ET buffer
            nc.gpsimd.tensor_tensor(out=MT, in0=EQ10, in1=IN3, op=OP.mult)
            wm = IN2[:, 11, :]
            VLD = pool.tile([P, FW], F16, tag="K24")
            nc.vector.tensor_scalar(out=VLD, in0=clsf, scalar1=-0.5,
                                    scalar2=None, op0=OP.is_ge)
            nc.scalar.activation(JUNK, wm, AF.Copy, accum_out=ACC[:, 8:9])
            nc.scalar.activation(JUNK, VLD, AF.Copy, accum_out=ACC[:, 1:2])
            L5 = pool.tile([P, 5, FW], F16, tag="S5a")
            nc.gpsimd.tensor_tensor(out=L5, in0=MT[:, 0:5, :], in1=MT[:, 5:10, :],
                                    op=OP.add)

            S2 = pool.tile([P, 2, FW], F16, tag="T2a")
            nc.vector.tensor_tensor(out=S2, in0=S5[:, 0:2, :], in1=S5[:, 2:4, :],
                                    op=OP.add)
            SSs = pool.tile([P, FW], F16, tag="K16")
            nc.vector.tensor_tensor(out=SSs, in0=S2[:, 0, :], in1=S2[:, 1, :],
                                    op=OP.add)
            nc.vector.tensor_tensor(out=SSs, in0=SSs, in1=S5[:, 4, :], op=OP.add)
            L2 = pool.tile([P, 2, FW], F16, tag="T2b")
            nc.vector.tensor_tensor(out=L2, in0=L5[:, 0:2, :], in1=L5[:, 2:4, :],
                                    op=OP.add)
            LT = pool.tile([P, FW], F16, tag="K17")
            nc.vector.tensor_tensor(out=LT, in0=L2[:, 0, :], in1=L2[:, 1, :],
                                    op=OP.add)
            nc.vector.tensor_tensor(out=LT, in0=LT, in1=L5[:, 4, :], op=OP.add)
            LNS = SSs
            nc.scalar.activation(LNS, SSs, AF.Ln)
            LPT = LT
            nc.vector.tensor_tensor(out=LPT, in0=LT, in1=LNS, op=OP.subtract)
            PTT = pool.tile([P, FW], F16, tag="K20")
            nc.scalar.activation(PTT, LPT, AF.Exp)
            OM2 = PTT
            nc.scalar.activation(OM2, PTT, AF.Square, scale=-1.0, bias=1.0)
            F1 = OM2
            nc.vector.tensor_tensor(out=F1, in0=OM2, in1=LPT, op=OP.mult)
            MPOS = pool.tile([P, FW], F16, tag="K23")
            nc.vector.tensor_scalar(out=MPOS, in0=clsf, scalar1=0.5,
                                    scalar2=None, op0=OP.is_gt)
            nc.vector.tensor_scalar(out=MPOS, in0=MPOS, scalar1=-0.5,
                                    scalar2=0.75, op0=OP.mult, op1=OP.add)
            nc.vector.tensor_tensor(out=F1, in0=F1, in1=MPOS, op=OP.mult)
            acc_sum(F1, VLD, 0, scale=-1.0)

            # ============ cross-partition reduce + output ============
            PS = ppool.tile([1, 16], F32)
            nc.tensor.matmul(PS, ones, ACC, start=True, stop=True)
            OUT = spool.tile([1, 16], F32)
            nc.scalar.copy(out=OUT, in_=PS)
            nc.sync.dma_start(out=outp[:, :], in_=OUT)
    nc.compile()
    nc._dbg_slots = dbg_slots
    return nc


_NC_CACHE = None


def _get_nc():
    global _NC_CACHE
    if _NC_CACHE is None:
        _NC_CACHE = build_bass()
    return _NC_CACHE


def pack_inputs(cls_pred, reg_pred, iou_pred, reg_targets, iou_targets,
                cls_targets, reg_weights):
    """Returns list of 8 per-core input dicts (in1/in2/in3 fp16 arrays)."""
    B = cls_pred.shape[0]
    maps = []
    for b in range(B):
        rp = np.asarray(reg_pred[b], np.float32).reshape(9, P, FW)
        rt = np.asarray(reg_targets[b], np.float32).reshape(9, P, FW)
        h1a = np.empty((2, P, FW), np.float16)
        h1a[0] = rp[6]; h1a[1] = rt[6]
        h1b = np.empty((8, P, FW), np.float16)
        h1b[0] = rp[3]; h1b[1] = rp[4]
        h1b[2] = rt[3]; h1b[3] = rt[4]
        h1b[4] = rp[0]; h1b[5] = rp[1]
        h1b[6] = rt[0]; h1b[7] = rt[1]
        h2 = np.empty((12, P, FW), np.float16)
        h2[0] = rp[2]; h2[1] = rt[2]
        h2[2] = rp[5]; h2[3] = rt[5]
        h2[4] = rp[7]; h2[5] = rt[7]
        h2[6] = rp[8]; h2[7] = rt[8]
        h2[8] = np.asarray(iou_pred[b], np.float32).reshape(P, FW)
        h2[9] = np.asarray(iou_targets[b], np.float32).reshape(P, FW)
        h2[10] = np.asarray(cls_targets[b]).astype(np.float32).reshape(P, FW)
        h2[11] = np.asarray(reg_weights[b]).astype(np.float32).reshape(P, FW)
        h3 = np.asarray(cls_pred[b], np.float32).reshape(10, P, FW).astype(np.float16)
        maps.append({
            "in1a": np.ascontiguousarray(h1a.transpose(1, 0, 2)),
            "in1b": np.ascontiguousarray(h1b.transpose(1, 0, 2)),
            "in2": np.ascontiguousarray(h2.transpose(1, 0, 2)),
            "in3": np.ascontiguousarray(h3.transpose(1, 0, 2)),
        })
    return maps


def combine(parts):
    """parts: [8, 1, 16] per-core raw sums -> final [7] float32."""
    p = np.asarray(parts, np.float64).sum(0).reshape(-1)
    focal_s, valid_s, diou_s, z_s, h_s, vx_s, vy_s, bce_s, w_s = p[:9]
    num_pos = max(w_s, 1.0)
    cls_loss = focal_s / max(valid_s, 1.0)
    bev_loss = (diou_s + w_s) / num_pos
    z_loss = (z_s - 0.5 * w_s) / num_pos
    h_loss = (h_s - 0.5 * w_s) / num_pos
    vel_loss = (vx_s + vy_s - w_s) / num_pos
    iou_loss = bce_s / num_pos
    total = cls_loss + 2.0 * bev_loss + z_loss + h_loss + vel_loss + iou_loss
    return np.array([total, cls_loss, bev_loss, z_loss, h_loss, vel_loss, iou_loss],
                    np.float32)


def kernel(cls_pred, reg_pred, iou_pred, reg_targets, iou_targets,
           cls_targets, reg_weights, _trace=False):
    cls_pred, reg_pred, iou_pred, reg_targets, iou_targets, cls_targets, reg_weights = (
        np.asarray(a) for a in (cls_pred, reg_pred, iou_pred, reg_targets,
                                iou_targets, cls_targets, reg_weights))
    nc = _get_nc()
    in_maps = pack_inputs(cls_pred, reg_pred, iou_pred, reg_targets,
                          iou_targets, cls_targets, reg_weights)
    res = run_bass_kernel_spmd(nc, in_maps, core_ids=list(range(8)), trace=_trace)
    parts = [res.results[i]["out"] for i in range(8)]
    out = combine(parts)
    if _trace:
        return out, res
    return out
